# revision 51
# baseline (speedup 1.0000x reference)
"""Trainium2 Bass kernel for a 4-layer bigram-LM dense transformer.

Full-model shapes: B=2, T=2048, E=256, H=8, L=4, V=32000.

Sharding over 8 NeuronCores (self-contained, hardcoded):
  - 2-way data parallel over batch: cores 0-3 handle batch 0, cores 4-7
    batch 1 (a "batch group" of 4 cores each).
  - Within a batch group, per-token work (LN / QKV / wo / FFN) is
    replicated; attention (the exp-heavy part) is sharded 2 heads/core
    and re-assembled with one 4-rank AllGather per layer (bf16 payload).
  - lm_head is sharded 4-way over vocab columns within the group
    (8000 cols/core, padded to 8192), so the dominant logits write is
    split 8 ways and emitted as fp16 (host converts back to fp32).

Compute layout: activations live transposed [E, T] in SBUF so every
matmul contracts over the partition axis with zero transposes. All
activations and weights are bf16 (PSUM accumulation stays fp32). LN
gains are folded into the consuming weights host-side; LN biases become
per-output constants. LN stats come from 1/E-scaled ones-matmuls: each
t-block's mean / mean-square land at a matmul-position-legal PSUM row
(even t-blocks rows {0,32}, odd {64,96} of one stats bank), so every
t-block's rstd chain (mean^2 on ACT Square, var on DVE, then
exp(-0.5*ln(var+eps)) on ACT - all functions in the single preloaded
act table set) reads 32-aligned rows and emits its scale|shift pair at
partition 0, where one gpsimd partition_broadcast per t-block fans both
across partitions (no PE broadcast matmuls, no ACT staging copies).
Chains are emitted right after their t-block's stats so they hide under
the next t-block's wo/FFN matmuls. gpsimd (Q7) cannot read PSUM, so it
only ever touches SBUF tiles.

Attention: q and k are produced by one merged 128-wide matmul per
(e, t-block). Scores/exp run per chunk-PAIR: both chunks' scores land
in one 2-bank PSUM tile and a single [128, 2, TB] exp converts them
(the later diagonal chunk's scores are extended down to the pair start
and re-masked with a [128, 2, 256] pair mask). att@V for t-blocks 1-3
runs in fp8e4m3 DoubleRow mode - one matmul per chunk-pair at half
cost; dual-fp8 ldweights requires a multiple-of-16 column count and a
partition-0 destination, so V tiles are 48 wide ([v(32)|ones|zeros])
and each head accumulates into its own PSUM bank. Token block 0
(queries < 512, small softmax support where fp8 quantization noise
doesn't average out) keeps the bf16 per-chunk path. Row-sums ride
along as the ones column; 1/rowsum is fanned across partitions by
partition_broadcast. The per-layer schedule is software-pipelined:
qkv/attention of layer l, AllGather, then wo+LN2 and FFN+LN1(l+1)
with per-t-block oT loads so wo unblocks before the full gather lands.

The lm_head matmul/copy pipeline rotates across all 8 PSUM banks and
its PSUM->SBUF fp16 copies alternate ACT/DVE so the engines stay under
the matmul. Logits are emitted fp16 in a [128, T/128, V-shard] layout
so eight token-chunks batch into each DMA.
"""

import numpy as np
import ml_dtypes

import concourse.bass as bass
import concourse.mybir as mybir
import concourse.tile as tile
from concourse import bacc
from concourse.bass_utils import run_bass_kernel_spmd

AF = mybir.ActivationFunctionType
ALU = mybir.AluOpType
FP32 = mybir.dt.float32
FP32R = mybir.dt.float32r
BF16 = mybir.dt.bfloat16
FP16 = mybir.dt.float16
FP8 = mybir.dt.float8e4
NP_BF16 = ml_dtypes.bfloat16
NP_FP8 = ml_dtypes.float8_e4m3
DR = mybir.MatmulPerfMode.DoubleRow

# model dims (full problem)
B, T, E, H, L, V = 2, 2048, 256, 8, 4, 32000
HD = E // H  # 32
EPS = 1e-5
NCORES = 8
GROUP = 4  # cores per batch group
HPC = H // GROUP  # heads per core (2)
VS = V // GROUP  # vocab shard per core (8000)
VSP = 8192  # padded vocab shard
TB = 512  # t-block (PSUM bank free dim)
SC = 128  # s-chunk (partition dim)
ET = E // 128  # embedding partition tiles (2)
FF = 4 * E  # 1024
UT = FF // 128  # ffn u-tiles (8)


def _act_table_id(arch):
    """Index of the one act-func-set that serves every ACT function we
    emit (Exp, Ln, Relu, Copy, Identity, Square)."""
    from concourse.hw_specs import get_activation_tables

    need = {AF.Exp, AF.Ln, AF.Relu, AF.Copy, AF.Identity, AF.Square}
    for i, (name, fns) in enumerate(get_activation_tables(arch).items()):
        if need <= fns:
            return i
    raise RuntimeError("no single act table covers required functions")


def build_nc(t=T, layers=L, vsp=VSP, use_collective=True):
    """Build + compile the per-core Bass program (SPMD: same program, 8 cores)."""
    nt = t // TB      # t-blocks
    nsc = t // SC     # s-chunks
    ntc = t // 128    # t-chunks for lm head
    nvb = vsp // 512  # vocab blocks

    nc = bacc.Bacc("TRN2", num_devices=NCORES)

    # ---- DRAM parameters (per core) ----
    x0 = nc.declare_dram_parameter("x0", [E, t], BF16, isOutput=False)
    wqkv = nc.declare_dram_parameter("wqkv", [layers, 128, ET, 6 * HD], BF16, isOutput=False)
    wo_p = nc.declare_dram_parameter("wo", [layers, 128, ET, E], BF16, isOutput=False)
    w1_p = nc.declare_dram_parameter("w1", [layers, 128, ET, FF], BF16, isOutput=False)
    w2_p = nc.declare_dram_parameter("w2", [layers, 128, UT, E], BF16, isOutput=False)
    vecs = nc.declare_dram_parameter("vecs", [layers, 128, 20], FP32, isOutput=False)
    whead = nc.declare_dram_parameter("whead", [128, ET, vsp], BF16, isOutput=False)
    # diag-pair mask: [:,0,:] = [tri | ones], [:,1,:] = [zeros | tri]
    mask2b_p = nc.declare_dram_parameter("mask2b", [SC, 2, 2 * SC], BF16, isOutput=False)
    mask28_p = nc.declare_dram_parameter("mask28", [SC, 2, 2 * SC], FP8, isOutput=False)
    peye = nc.declare_dram_parameter("peye", [128, nt, nt], BF16, isOutput=False)
    logits = nc.declare_dram_parameter("logits", [128, t // 128, vsp], FP16, isOutput=True)

    # internal DRAM bounce buffers for the per-layer AllGather (bf16)
    cc_in = [nc.dram_tensor(f"cc_in{l}", [HPC * HD, t], BF16) for l in range(layers)]
    cc_out = [nc.dram_tensor(f"cc_out{l}", [GROUP * HPC * HD, t], BF16) for l in range(layers)]
    groups = [[0, 1, 2, 3], [4, 5, 6, 7]]

    from contextlib import ExitStack
    with tile.TileContext(nc) as tc:
        with ExitStack() as _ctx:
            persist = _ctx.enter_context(tc.tile_pool(name="persist", bufs=1))
            wpool2 = _ctx.enter_context(tc.tile_pool(name="wpool2", bufs=2))
            wpool1 = _ctx.enter_context(tc.tile_pool(name="wpool1", bufs=2))
            actp = _ctx.enter_context(tc.tile_pool(name="actp", bufs=1))
            xlnp = _ctx.enter_context(tc.tile_pool(name="xlnp", bufs=3))
            bigp = _ctx.enter_context(tc.tile_pool(name="bigp", bufs=3))
            expp = _ctx.enter_context(tc.tile_pool(name="expp", bufs=3))
            smallp = _ctx.enter_context(tc.tile_pool(name="smallp", bufs=3))
            tmpp = _ctx.enter_context(tc.tile_pool(name="tmpp", bufs=3))
            lgp = _ctx.enter_context(tc.tile_pool(name="lgp", bufs=3))
            whp = _ctx.enter_context(tc.tile_pool(name="whp", bufs=16))
            # PSUM: 'pair' 2x[128,2,TB] (4 banks) + 'mm' 2x (2) + 'op' 2x (2)
            ps_pair = _ctx.enter_context(tc.tile_pool(name="ps_pair", bufs=2, space="PSUM"))
            ps_mm = _ctx.enter_context(tc.tile_pool(name="ps_mm", bufs=2, space="PSUM"))
            ps_op = _ctx.enter_context(tc.tile_pool(name="ps_op", bufs=2, space="PSUM"))

            # single act-table preload: every later activation finds its
            # function already resident, so the compile pass inserts no
            # further loads
            nc.scalar.add_instruction(
                mybir.InstLoadActFuncSet(
                    name=nc.get_next_instruction_name(),
                    ins=[],
                    outs=[],
                    act_func_set_id=_act_table_id(nc.m.arch),
                )
            )

            # ---- persistent tiles ----
            xT = [persist.tile([128, t], BF16, tag=f"xT{e}", name=f"xT{e}") for e in range(ET)]
            for e in range(ET):
                nc.sync.dma_start(out=xT[e], in_=x0[128 * e : 128 * (e + 1), :])
            mask2b = persist.tile([SC, 2, 2 * SC], BF16, tag="mask2b")
            nc.sync.dma_start(out=mask2b, in_=mask2b_p[:, :, :])
            mask28 = persist.tile([SC, 2, 2 * SC], FP8, tag="mask28")
            nc.sync.dma_start(out=mask28, in_=mask28_p[:, :, :])
            # v tiles: per chunk cols = [vA(32) | ones | vB(32) | ones]; the
            # ones column makes row 32 of the AV output the softmax row-sum.
            # vt8: all chunks (fp8, DoubleRow path for t-blocks >= 1);
            # vtb: chunks 0-3 (bf16 path for t-block 0).
            # fp8 v tiles are 48 cols wide (dual-fp8 ldweights requires a
            # multiple of 16): [v(32) | ones | zeros(15)]
            VW = 48
            vt8s, vtbs = [], []
            for par in range(2):
                vt8 = persist.tile([128, nsc, 2, VW], FP8, tag=f"vt8_{par}", name=f"vt8_{par}")
                vtb = persist.tile([128, 4, 2, HD + 1], BF16, tag=f"vtb_{par}", name=f"vtb_{par}")
                nc.vector.memset(vt8[:, :, :, :], 0.0)
                nc.vector.memset(vt8[:, :, :, HD : HD + 1], 1.0)
                nc.vector.memset(vtb[:, :, :, HD : HD + 1], 1.0)
                vt8s.append(vt8)
                vtbs.append(vtb)
            # eyeblk holds 1/E so the stats matmuls yield mean and mean-square
            eyeblk = persist.tile([128, nt, nt], BF16, tag="eyeblk")
            nc.sync.dma_start(out=eyeblk, in_=peye[:, :, :])
            # own heads' normalized attention out, pre-AllGather, [32, t] each
            oTp = [persist.tile([HD, t], BF16, tag=f"oTp{h}", name=f"oTp{h}") for h in range(HPC)]
            epst = persist.tile([128, 1], FP32, tag="epst")
            nc.vector.memset(epst, EPS)

            def ln_stats_tb(src, st, tb):
                """x and x^2 1/E-sums for t-block tb. Even t-blocks use PSUM
                rows {0, 32}, odd rows {64, 96} (all matmul-position-legal),
                so each t-block's rstd chain reads 32-aligned rows and its
                outputs live at partition 0 for partition_broadcast."""
                r = tb % 2
                tbl = slice(TB * tb, TB * (tb + 1))
                for e in range(ET):
                    sq = bigp.tile([128, TB], BF16, tag="sq", name="sq")
                    if e == 0:
                        nc.vector.tensor_tensor(
                            out=sq, in0=src[e][:, tbl], in1=src[e][:, tbl], op=ALU.mult
                        )
                    else:
                        nc.scalar.activation(out=sq, in_=src[e][:, tbl], func=AF.Square)
                    nc.tensor.matmul(
                        st[64 * r : 64 * r + 1, :], eyeblk[:, tb, tb : tb + 1],
                        src[e][:, tbl],
                        start=(e == 0), stop=(e == ET - 1),
                        tile_position=(0, 64 * r),
                    )
                    nc.tensor.matmul(
                        st[32 + 64 * r : 33 + 64 * r, :], eyeblk[:, tb, tb : tb + 1],
                        sq,
                        start=(e == 0), stop=(e == ET - 1),
                        tile_position=(0, 32 + 64 * r),
                    )

            def ln_chain_apply_tb(src, st, tb, out_tiles):
                """rstd chain + scale/shift broadcast + apply for one t-block."""
                r = tb % 2
                var1 = smallp.tile([1, TB], FP32, tag="var4", name="var1")
                # mean^2 on ACT (a TensorTensor may read only ONE input from
                # PSUM, and both would be the stats bank here)
                nc.scalar.activation(
                    out=var1, in_=st[64 * r : 64 * r + 1, :], func=AF.Square
                )
                nc.vector.tensor_tensor(
                    out=var1, in0=st[32 + 64 * r : 33 + 64 * r, :], in1=var1,
                    op=ALU.subtract,
                )
                nc.scalar.activation(out=var1, in_=var1, func=AF.Ln, bias=epst[0:1, :])
                smt = smallp.tile([1, 2, TB], BF16, tag="smt", name="smt")
                nc.scalar.activation(out=smt[:, 0, :], in_=var1, func=AF.Exp, scale=-0.5)
                nc.vector.scalar_tensor_tensor(
                    out=smt[:, 1, :], in0=st[64 * r : 64 * r + 1, :], scalar=-1.0,
                    in1=smt[:, 0, :], op0=ALU.mult, op1=ALU.mult,
                )
                bc = tmpp.tile([128, 2, TB], BF16, tag="bc", name="bc")
                nc.gpsimd.partition_broadcast(bc[:, :, :], smt[0:1, :, :])
                for e in range(ET):
                    eng = nc.vector
                    tmp = bigp.tile([128, TB], BF16, tag="lnt2", name="lntmp2")
                    eng.tensor_tensor(
                        out=tmp, in0=src[e][:, TB * tb : TB * (tb + 1)],
                        in1=bc[:, 0, :], op=ALU.mult,
                    )
                    eng.tensor_tensor(
                        out=out_tiles[e][:, TB * tb : TB * (tb + 1)],
                        in0=tmp, in1=bc[:, 1, :], op=ALU.add,
                    )

            def layernorm(src, out_tiles):
                """Standalone per-half LN (used for layer 0's LN1)."""
                for half in range(2):
                    st = ps_mm.tile([98, TB], FP32, tag="mm", name="st")
                    for tb in (2 * half, 2 * half + 1):
                        ln_stats_tb(src, st, tb)
                        ln_chain_apply_tb(src, st, tb, out_tiles)

            # ---------- per-piece emitters ----------
            W = {}  # layer -> weight tiles

            def prefetch_weights(l):
                wq_t = [wpool2.tile([128, 6 * HD], BF16, tag=f"wqkv{e}", name=f"wqkv{e}") for e in range(ET)]
                wo_t = [wpool2.tile([128, E], BF16, tag=f"wo{e}", name=f"wot{e}") for e in range(ET)]
                w1_t = [wpool1.tile([128, FF], BF16, tag=f"w1{e}", name=f"w1t{e}") for e in range(ET)]
                w2_t = wpool1.tile([128, UT, E], BF16, tag="w2")
                vec = wpool2.tile([128, 20], FP32, tag="vec")
                for e in range(ET):
                    nc.sync.dma_start(out=wq_t[e], in_=wqkv[l, :, e, :])
                    nc.sync.dma_start(out=wo_t[e], in_=wo_p[l, :, e, :])
                    nc.sync.dma_start(out=w1_t[e], in_=w1_p[l, :, e, :])
                nc.sync.dma_start(out=w2_t, in_=w2_p[l, :, :, :])
                nc.sync.dma_start(out=vec, in_=vecs[l, :, :])
                W[l] = dict(wq=wq_t, wo=wo_t, w1=w1_t, w2=w2_t, vec=vec)

            def qkv_attn_tb(l, tb, xln, qT, kT, vt8, vtb):
                wq_t, vec = W[l]["wq"], W[l]["vec"]
                tsl = slice(TB * tb, TB * (tb + 1))
                # merged q|k projection: one 128-wide output per e
                qk = ps_mm.tile([128, TB], FP32, tag="mm", name="qk")
                for e in range(ET):
                    nc.tensor.matmul(
                        qk[:, :], wq_t[e][:, 0 : 4 * HD], xln[e][:, tsl],
                        start=(e == 0), stop=(e == ET - 1),
                    )
                # +cq/+ck: the ln1_b contributions, folded host-side
                nc.vector.tensor_scalar(
                    qT[:, tsl], qk[0 : 2 * HD, :], vec[0 : 2 * HD, 0:1], None, ALU.add
                )
                nc.scalar.activation(
                    out=kT[:, tsl], in_=qk[2 * HD : 4 * HD, :], func=AF.Identity,
                    bias=vec[2 * HD : 4 * HD, 0:1],
                )
                for i in range(4 * tb, 4 * tb + 4):
                    # vp lives in the 'mm' tag so op_ps (long-lived per
                    # t-block) only rotates against itself
                    vp = ps_mm.tile([128, 2, HD], FP32, tag="mm", name="vp")
                    for e in range(ET):
                        nc.tensor.matmul(
                            vp[:, :, :],
                            xln[e][:, SC * i : SC * (i + 1)],
                            wq_t[e][:, 4 * HD : 6 * HD],
                            start=(e == 0), stop=(e == ET - 1),
                        )
                    nc.vector.tensor_copy(out=vt8[:, i, :, 0:HD], in_=vp[:, :, :])
                    if i < 4:
                        nc.scalar.copy(out=vtb[:, i, :, 0:HD], in_=vp[:, :, :])

                # ---- attention for this t-block ----
                fp8blk = tb >= 1
                npair = 2 * tb + 2
                # one PSUM bank per head: dual-fp8 matmul output must start
                # at partition 0
                op_h = [ps_op.tile([128, TB], FP32, tag="op", name=f"op_h{h}")
                        for h in range(HPC)]

                def emit_av(j, h, exh, dp):
                    if fp8blk:
                        nc.tensor.matmul(
                            op_h[h][0:VW, dp:TB],
                            vt8[:, 2 * j : 2 * j + 2, h, :],
                            exh[:, :, dp:TB],
                            start=(j == 0), stop=(j == npair - 1),
                            perf_mode=DR,
                            tile_position=(0, 0),
                        )
                    else:
                        for jj in range(2):
                            nc.tensor.matmul(
                                op_h[h][0 : HD + 1, dp:TB],
                                vtb[:, 2 * j + jj, h, :],
                                exh[:, jj, dp:TB],
                                start=(j == 0 and jj == 0),
                                stop=(j == npair - 1 and jj == 1),
                                tile_position=(0, 0),
                            )

                exdt = FP8 if fp8blk else BF16
                m2 = mask28 if fp8blk else mask2b
                pend = []  # FIFO; AV trails exp by one pair
                for j in range(npair):
                    dp = max(0, SC * 2 * j - TB * tb)
                    psl = slice(dp, TB)
                    tgl = slice(TB * tb + dp, TB * (tb + 1))
                    for h in range(HPC):
                        rsl = slice(32 * h, 32 * (h + 1))
                        at_ps = ps_pair.tile([128, 2, TB], FP32, tag="pair", name="at_ps")
                        exh = expp.tile(
                            [128, 2, TB], exdt,
                            tag=f"exp{h}{'8' if fp8blk else 'b'}", name="exh",
                        )
                        for jj in range(2):
                            i = 2 * j + jj
                            nc.tensor.matmul(
                                at_ps[:, jj, psl],
                                kT[rsl, SC * i : SC * (i + 1)],
                                qT[rsl, tgl],
                                start=True, stop=True,
                                tile_position=(32 * h, 0),
                            )
                        nc.scalar.activation(
                            out=exh[:, :, psl], in_=at_ps[:, :, psl],
                            func=AF.Exp, scale=float(E) ** -0.5,
                        )
                        if j >= 2 * tb:  # diagonal pair: mask both chunks
                            nc.vector.tensor_tensor(
                                out=exh[:, :, dp : dp + 2 * SC],
                                in0=exh[:, :, dp : dp + 2 * SC],
                                in1=m2[:, :, :], op=ALU.mult,
                            )
                        pend.append((j, h, exh, dp))
                    while len(pend) > HPC:
                        emit_av(*pend.pop(0))
                for ent in pend:
                    emit_av(*ent)
                # normalize each head by its row-sum (psum row 64h+32):
                # bf16 reciprocal -> partition_broadcast -> multiply
                for h in range(HPC):
                    rr = smallp.tile([1, TB], BF16, tag=f"rr{h}", name="rr")
                    with nc.allow_low_precision(reason="1/rowsum applied in bf16"):
                        nc.vector.reciprocal(
                            out=rr[0:1, :],
                            in_=op_h[h][HD : HD + 1, :],
                        )
                    rb = tmpp.tile([HD, TB], BF16, tag="rbc", name="rb")
                    nc.gpsimd.partition_broadcast(rb[:, :], rr[0:1, :])
                    nc.vector.tensor_tensor(
                        out=oTp[h][:, TB * tb : TB * (tb + 1)],
                        in0=op_h[h][0:HD, :],
                        in1=rb,
                        op=ALU.mult,
                    )
                    # stream this t-block's slice to the AllGather bounce
                    # buffer now, so the collective input is ready the
                    # moment the last block finishes
                    nc.sync.dma_start(
                        out=cc_in[l][HD * h : HD * (h + 1), TB * tb : TB * (tb + 1)],
                        in_=oTp[h][:, TB * tb : TB * (tb + 1)],
                    )

            def wo_ln2(l, oT, xln2):
                """wo projection + residual, interleaved with the LN2 stats
                so the rstd chains hide under the following t-blocks' PE."""
                wo_t, vec = W[l]["wo"], W[l]["vec"]
                for half in range(2):
                    st = ps_mm.tile([98, TB], FP32, tag="mm", name="st2")
                    for tb in (2 * half, 2 * half + 1):
                        tsl = slice(TB * tb, TB * (tb + 1))
                        for eo in range(ET):
                            wp = ps_op.tile([128, TB], FP32, tag="op", name="wp")
                            for e in range(ET):
                                nc.tensor.matmul(
                                    wp[:, :],
                                    wo_t[e][:, 128 * eo : 128 * (eo + 1)],
                                    oT[e][:, tsl],
                                    start=(e == 0), stop=(e == ET - 1),
                                )
                            nc.vector.scalar_tensor_tensor(
                                out=xT[eo][:, tsl], in0=wp[:, :],
                                scalar=vec[:, 8 + eo : 9 + eo], in1=xT[eo][:, tsl],
                                op0=ALU.add, op1=ALU.add,
                            )
                        ln_stats_tb(xT, st, tb)
                        ln_chain_apply_tb(xT, st, tb, xln2)

            def ffn_lnnext(l, xln2, xnext):
                """FFN + residual, interleaved with the next LN's stats; the
                chains run while the other half's FFN matmuls execute."""
                w1_t, w2_t, vec = W[l]["w1"], W[l]["w2"], W[l]["vec"]
                for half in range(2):
                    st = ps_mm.tile([98, TB], FP32, tag="mm", name="st1")
                    for tb in (2 * half, 2 * half + 1):
                        tsl = slice(TB * tb, TB * (tb + 1))
                        ru_halves = []
                        for rh in range(2):
                            ru = bigp.tile([128, UT // 2, TB], BF16, tag="big", name="ru")
                            for uu in range(UT // 2):
                                ut = rh * (UT // 2) + uu
                                up = ps_op.tile([128, TB], FP32, tag="op", name="up")
                                for e in range(ET):
                                    nc.tensor.matmul(
                                        up[:, :],
                                        w1_t[e][:, 128 * ut : 128 * (ut + 1)],
                                        xln2[e][:, tsl],
                                        start=(e == 0), stop=(e == ET - 1),
                                    )
                                # relu+bias: split across ACT and Pool
                                if ut % 2 == 0:
                                    nc.scalar.activation(
                                        out=ru[:, uu, :], in_=up[:, :], func=AF.Relu,
                                        bias=vec[:, 10 + ut : 11 + ut],
                                    )
                                else:
                                    nc.vector.tensor_scalar(
                                        ru[:, uu, :], up[:, :],
                                        vec[:, 10 + ut : 11 + ut], 0.0,
                                        ALU.add, ALU.max,
                                    )
                            ru_halves.append(ru)
                        wp2_pair = ps_pair.tile([128, 2, TB], FP32, tag="pair", name="wp2")
                        for eo in range(ET):
                            wp2 = wp2_pair[:, eo % 2, :]
                            for ut in range(UT):
                                nc.tensor.matmul(
                                    wp2,
                                    w2_t[:, ut, 128 * eo : 128 * (eo + 1)],
                                    ru_halves[ut // (UT // 2)][:, ut % (UT // 2), :],
                                    start=(ut == 0), stop=(ut == UT - 1),
                                )
                            nc.vector.scalar_tensor_tensor(
                                out=xT[eo][:, tsl], in0=wp2,
                                scalar=vec[:, 18 + eo : 19 + eo], in1=xT[eo][:, tsl],
                                op0=ALU.add, op1=ALU.add,
                            )
                        ln_stats_tb(xT, st, tb)
                        ln_chain_apply_tb(xT, st, tb, xnext)

            GB = min(8, ntc)  # token-chunks batched per logits DMA
            whs = []

            def prefetch_whead():
                for vb in range(nvb):
                    wh = whp.tile([128, ET, 512], BF16, tag="wh", name=f"wh{vb}")
                    nc.sync.dma_start(out=wh, in_=whead[:, :, 512 * vb : 512 * (vb + 1)])
                    whs.append(wh)

            def lm_group(g, xf):
                for vb in range(nvb):
                    wh = whs[vb]
                    lg = lgp.tile([128, GB, 512], FP16, tag="lg")
                    lpp = None
                    for k in range(GB):
                        tcn = GB * g + k
                        # rotate lp across all 8 PSUM banks: mm, op, and the
                        # two halves of an (idle) attention pair tile
                        sel = k % 4
                        if sel == 0:
                            lp = ps_mm.tile([128, 512], FP32, tag="mm", name="lp")
                        elif sel == 1:
                            lp = ps_op.tile([128, 512], FP32, tag="op", name="lp")
                        else:
                            if sel == 2:
                                lpp = ps_pair.tile([128, 2, TB], FP32, tag="pair", name="lpp")
                            lp = lpp[:, sel - 2, :]
                        for e in range(ET):
                            nc.tensor.matmul(
                                lp,
                                xf[e][:, 128 * tcn : 128 * (tcn + 1)],
                                wh[:, e, :],
                                start=(e == 0), stop=(e == ET - 1),
                            )
                        if (vb + tcn) % 2 == 0:
                            nc.vector.tensor_copy(out=lg[:, k, :], in_=lp)
                        else:
                            nc.scalar.copy(out=lg[:, k, :], in_=lp)
                    nc.sync.dma_start(
                        out=logits[:, GB * g : GB * (g + 1), 512 * vb : 512 * (vb + 1)],
                        in_=lg,
                    )

            # ================= layers (software-pipelined) =================
            prefetch_weights(0)
            xln_cur = [xlnp.tile([128, t], BF16, tag=f"xln{e}", name=f"xln{e}") for e in range(ET)]
            layernorm(xT, out_tiles=xln_cur)
            for l in range(layers):
                if l + 1 < layers:
                    prefetch_weights(l + 1)
                else:
                    # lm_head weights stream in during the last layer so the
                    # first logits matmuls never wait on DMA
                    prefetch_whead()
                qT = actp.tile([2 * HD, t], BF16, tag="qT")
                kT = actp.tile([2 * HD, t], BF16, tag="kT")
                vt8, vtb = vt8s[l % 2], vtbs[l % 2]
                for tb in range(nt):
                    qkv_attn_tb(l, tb, xln_cur, qT, kT, vt8, vtb)

                # ---- AllGather heads across the 4-core batch group ----
                oT = [actp.tile([128, t], BF16, tag=tg, name=f"oT_{tg}") for tg in ("qT", "kT")]
                if use_collective:
                    nc.gpsimd.collective_compute(
                        "AllGather", ALU.bypass,
                        replica_groups=groups,
                        ins=[cc_in[l][:, :]], outs=[cc_out[l][:, :]],
                    )
                # per-t-block loads so wo(tb0) unblocks before the full
                # gather output lands (no-collective build: same DMAs,
                # timing-only)
                for e in range(ET):
                    for tb in range(nt):
                        nc.sync.dma_start(
                            out=oT[e][:, TB * tb : TB * (tb + 1)],
                            in_=cc_out[l][128 * e : 128 * (e + 1), TB * tb : TB * (tb + 1)],
                        )

                xln2 = [xlnp.tile([128, t], BF16, tag=f"xln{e}", name=f"xln{e}") for e in range(ET)]
                wo_ln2(l, oT, xln2)
                xnext = [xlnp.tile([128, t], BF16, tag=f"xln{e}", name=f"xln{e}") for e in range(ET)]
                ffn_lnnext(l, xln2, xnext)
                xln_cur = xnext

            # ================= lm_head =================
            lm_group(0, xln_cur)
            lm_group(1, xln_cur)
    nc.compile()
    return nc


# ---------------- host-side prep / unshard ----------------

def prep_core_inputs(c, X, tok_emb, pos_emb, wq, wk, wv, wo, bo, w1, b1, w2, b2,
                     ln1_g, ln1_b, ln2_g, ln2_b, lnf_g, lnf_b, w_head, b_head,
                     t=T, layers=L, vsp=VSP):
    b = c // GROUP
    j = c % GROUP
    heads = [HPC * j + k for k in range(HPC)]

    f32 = np.float32
    Xb = np.asarray(X[b]).astype(np.int64)
    x0 = (np.asarray(tok_emb)[Xb] + np.asarray(pos_emb)[:t]).astype(f32).T  # [E, t]

    wq = np.asarray(wq); wk = np.asarray(wk); wv = np.asarray(wv)
    wqkv_h = np.empty((layers, 128, ET, 6 * HD), f32)
    wo_h = np.empty((layers, 128, ET, E), f32)
    w1_h = np.empty((layers, 128, ET, FF), f32)
    w2_h = np.empty((layers, 128, UT, E), f32)
    vecs_h = np.zeros((layers, 128, 20), f32)
    for l in range(layers):
        # fold LN gains into the consuming weights and LN biases into
        # per-output-constant corrections (exact for affine LN):
        #   xln_true = xln_raw * g + b  =>  W^T xln_true = (gW)^T xln_raw + W^T b
        g1 = np.asarray(ln1_g[l]).astype(f32)[:, None]
        b1n = np.asarray(ln1_b[l]).astype(f32)
        g2 = np.asarray(ln2_g[l]).astype(f32)[:, None]
        b2n = np.asarray(ln2_b[l]).astype(f32)
        qc = np.concatenate([wq[l, h] for h in heads], axis=1)  # [E, 64]
        kc = np.concatenate([wk[l, h] for h in heads], axis=1)
        vc = np.concatenate([wv[l, h] for h in heads], axis=1)
        cq = qc.T @ b1n  # [64] q bias from ln1_b
        ck = kc.T @ b1n
        # v bias from ln1_b for ALL heads, folded through wo into bo
        cv_full = np.concatenate([wv[l, h].T @ b1n for h in range(H)])  # [E]
        bo_eff = np.asarray(bo[l]).astype(f32) + np.asarray(wo[l]).T @ cv_full
        b1_eff = np.asarray(b1[l]).astype(f32) + np.asarray(w1[l]).T @ b2n
        qkv = np.concatenate([qc, kc, vc], axis=1) * g1  # [E, 192]
        wqkv_h[l] = qkv.reshape(ET, 128, 6 * HD).transpose(1, 0, 2)
        wo_h[l] = np.asarray(wo[l]).reshape(ET, 128, E).transpose(1, 0, 2)
        w1_h[l] = (np.asarray(w1[l]) * g2).reshape(ET, 128, FF).transpose(1, 0, 2)
        w2_h[l] = np.asarray(w2[l]).reshape(UT, 128, E).transpose(1, 0, 2)
        vecs_h[l, 0 : 2 * HD, 0] = cq
        vecs_h[l, 2 * HD : 4 * HD, 0] = ck
        vecs_h[l, :, 8:10] = bo_eff.reshape(2, 128).T
        vecs_h[l, :, 10:18] = b1_eff.reshape(8, 128).T
        vecs_h[l, :, 18:20] = np.asarray(b2[l]).astype(f32).reshape(2, 128).T

    w_head = np.asarray(w_head) * np.asarray(lnf_g).astype(f32)[:, None]
    vs = w_head.shape[1] // GROUP
    wh = np.zeros((E, vsp), f32)
    wh[:, :vs] = w_head[:, vs * j : vs * (j + 1)]
    whead_h = np.ascontiguousarray(wh.reshape(ET, 128, vsp).transpose(1, 0, 2))

    # diag-pair mask [s, 2, 2*SC]: chunk-lo gets [tri | ones], chunk-hi
    # (one chunk later) gets [zeros | tri]
    sp = np.arange(SC)[:, None]
    cp = np.arange(2 * SC)[None, :]
    mask2 = np.zeros((SC, 2, 2 * SC), f32)
    mask2[:, 0, :] = (sp <= cp).astype(f32)
    mask2[:, 1, :] = (sp <= cp - SC).astype(f32)

    nt = t // TB
    peye_h = np.zeros((128, nt, nt), f32)
    for tb in range(nt):
        peye_h[:, tb, tb] = 1.0 / E

    bf = NP_BF16
    return {
        "x0": np.ascontiguousarray(x0).astype(bf),
        "wqkv": np.ascontiguousarray(wqkv_h).astype(bf),
        "wo": np.ascontiguousarray(wo_h).astype(bf),
        "w1": np.ascontiguousarray(w1_h).astype(bf),
        "w2": np.ascontiguousarray(w2_h).astype(bf),
        "vecs": np.ascontiguousarray(vecs_h),
        "whead": whead_h.astype(bf),
        "mask2b": mask2.astype(bf),
        "mask28": mask2.astype(NP_FP8),
        "peye": peye_h.astype(bf),
    }


_NC_CACHE = {}


def _get_nc():
    if "nc" not in _NC_CACHE:
        _NC_CACHE["nc"] = build_nc()
    return _NC_CACHE["nc"]


def kernel(**inputs):
    nc = _get_nc()
    in_maps = [prep_core_inputs(c, **inputs) for c in range(NCORES)]
    res = run_bass_kernel_spmd(nc, in_maps, list(range(NCORES)))
    out = np.empty((B, T, V), np.float32)
    for c in range(NCORES):
        b, j = c // GROUP, c % GROUP
        lg = res.results[c]["logits"]  # [128, T//128, VSP]
        lg = lg.transpose(1, 0, 2).reshape(T, VSP)
        out[b, :, VS * j : VS * (j + 1)] = lg[:, :VS].astype(np.float32)
    # b_head plus the final-LN bias folded through w_head (host-side)
    bh_eff = np.asarray(inputs["b_head"]).astype(np.float32) + (
        np.asarray(inputs["w_head"]).astype(np.float32).T
        @ np.asarray(inputs["lnf_b"]).astype(np.float32)
    )
    if np.any(bh_eff):
        out += bh_eff[None, None, :]
    return out


# revision 54
# speedup vs baseline: 1.0111x; 1.0111x over previous
"""Trainium2 Bass kernel for a 4-layer bigram-LM dense transformer.

Full-model shapes: B=2, T=2048, E=256, H=8, L=4, V=32000.

Sharding over 8 NeuronCores (self-contained, hardcoded):
  - 2-way data parallel over batch: cores 0-3 handle batch 0, cores 4-7
    batch 1 (a "batch group" of 4 cores each).
  - Within a batch group, per-token work (LN / QKV / wo / FFN) is
    replicated; attention (the exp-heavy part) is sharded 2 heads/core
    and re-assembled with one 4-rank AllGather per layer (bf16 payload).
  - lm_head is sharded 4-way over vocab columns within the group
    (8000 cols/core, padded to 8192), so the dominant logits write is
    split 8 ways and emitted as fp16 (host converts back to fp32).

Compute layout: activations live transposed [E, T] in SBUF so every
matmul contracts over the partition axis with zero transposes. All
activations and weights are bf16 (PSUM accumulation stays fp32). LN
gains are folded into the consuming weights host-side; LN biases become
per-output constants. LN stats come from 1/E-scaled ones-matmuls: each
t-block's mean / mean-square land at a matmul-position-legal PSUM row
(even t-blocks rows {0,32}, odd {64,96} of one stats bank), so every
t-block's rstd chain (mean^2 on ACT Square, var on DVE, then
exp(-0.5*ln(var+eps)) on ACT - all functions in the single preloaded
act table set) reads 32-aligned rows and emits its scale|shift pair at
partition 0, where one gpsimd partition_broadcast per t-block fans both
across partitions (no PE broadcast matmuls, no ACT staging copies).
Chains are emitted right after their t-block's stats so they hide under
the next t-block's wo/FFN matmuls. gpsimd (Q7) cannot read PSUM, so it
only ever touches SBUF tiles.

Attention: q and k are produced by one merged 128-wide matmul per
(e, t-block). Scores/exp run per chunk-PAIR: both chunks' scores land
in one 2-bank PSUM tile and a single [128, 2, TB] exp converts them
(the later diagonal chunk's scores are extended down to the pair start
and re-masked with a [128, 2, 256] pair mask). att@V for t-blocks 1-3
runs in fp8e4m3 DoubleRow mode - one matmul per chunk-pair at half
cost; dual-fp8 ldweights requires a multiple-of-16 column count and a
partition-0 destination, so V tiles are 48 wide ([v(32)|ones|zeros])
and each head accumulates into its own PSUM bank. Token block 0
(queries < 512, small softmax support where fp8 quantization noise
doesn't average out) keeps the bf16 per-chunk path. Row-sums ride
along as the ones column; 1/rowsum is fanned across partitions by
partition_broadcast. The per-layer schedule is software-pipelined:
qkv/attention of layer l, AllGather, then wo+LN2 and FFN+LN1(l+1)
with per-t-block oT loads so wo unblocks before the full gather lands.

The lm_head matmul/copy pipeline rotates across all 8 PSUM banks and
its PSUM->SBUF fp16 copies alternate ACT/DVE so the engines stay under
the matmul. Logits are emitted fp16 in a [128, T/128, V-shard] layout
so eight token-chunks batch into each DMA.
"""

import numpy as np
import ml_dtypes

import concourse.bass as bass
import concourse.mybir as mybir
import concourse.tile as tile
from concourse import bacc
from concourse.bass_utils import run_bass_kernel_spmd

AF = mybir.ActivationFunctionType
ALU = mybir.AluOpType
FP32 = mybir.dt.float32
FP32R = mybir.dt.float32r
BF16 = mybir.dt.bfloat16
FP16 = mybir.dt.float16
FP8 = mybir.dt.float8e4
NP_BF16 = ml_dtypes.bfloat16
NP_FP8 = ml_dtypes.float8_e4m3
DR = mybir.MatmulPerfMode.DoubleRow

# model dims (full problem)
B, T, E, H, L, V = 2, 2048, 256, 8, 4, 32000
HD = E // H  # 32
EPS = 1e-5
NCORES = 8
GROUP = 4  # cores per batch group
HPC = H // GROUP  # heads per core (2)
VS = V // GROUP  # vocab shard per core (8000)
VSP = 8192  # padded vocab shard
TB = 512  # t-block (PSUM bank free dim)
SC = 128  # s-chunk (partition dim)
ET = E // 128  # embedding partition tiles (2)
FF = 4 * E  # 1024
UT = FF // 128  # ffn u-tiles (8)


def _act_table_id(arch):
    """Index of the one act-func-set that serves every ACT function we
    emit (Exp, Ln, Relu, Copy, Identity, Square)."""
    from concourse.hw_specs import get_activation_tables

    need = {AF.Exp, AF.Ln, AF.Relu, AF.Copy, AF.Identity, AF.Square}
    for i, (name, fns) in enumerate(get_activation_tables(arch).items()):
        if need <= fns:
            return i
    raise RuntimeError("no single act table covers required functions")


def build_nc(t=T, layers=L, vsp=VSP, use_collective=True):
    """Build + compile the per-core Bass program (SPMD: same program, 8 cores)."""
    nt = t // TB      # t-blocks
    nsc = t // SC     # s-chunks
    ntc = t // 128    # t-chunks for lm head
    nvb = vsp // 512  # vocab blocks

    nc = bacc.Bacc("TRN2", num_devices=NCORES)

    # ---- DRAM parameters (per core) ----
    x0 = nc.declare_dram_parameter("x0", [E, t], BF16, isOutput=False)
    wqkv = nc.declare_dram_parameter("wqkv", [layers, 128, ET, 6 * HD], BF16, isOutput=False)
    wo_p = nc.declare_dram_parameter("wo", [layers, 128, ET, E], BF16, isOutput=False)
    w1_p = nc.declare_dram_parameter("w1", [layers, 128, ET, FF], BF16, isOutput=False)
    w2_p = nc.declare_dram_parameter("w2", [layers, 128, UT, E], BF16, isOutput=False)
    vecs = nc.declare_dram_parameter("vecs", [layers, 128, 20], FP32, isOutput=False)
    whead = nc.declare_dram_parameter("whead", [128, ET, vsp], BF16, isOutput=False)
    # diag-pair mask: [:,0,:] = [tri | ones], [:,1,:] = [zeros | tri]
    mask2b_p = nc.declare_dram_parameter("mask2b", [SC, 2, 2 * SC], BF16, isOutput=False)
    mask28_p = nc.declare_dram_parameter("mask28", [SC, 2, 2 * SC], FP8, isOutput=False)
    peye = nc.declare_dram_parameter("peye", [128, nt, nt], BF16, isOutput=False)
    logits = nc.declare_dram_parameter("logits", [128, t // 128, vsp], FP16, isOutput=True)

    # internal DRAM bounce buffers for the per-layer AllGather (bf16)
    cc_in = [nc.dram_tensor(f"cc_in{l}", [HPC * HD, t], BF16) for l in range(layers)]
    cc_out = [nc.dram_tensor(f"cc_out{l}", [GROUP * HPC * HD, t], BF16) for l in range(layers)]
    groups = [[0, 1, 2, 3], [4, 5, 6, 7]]

    from contextlib import ExitStack
    with tile.TileContext(nc) as tc:
        with ExitStack() as _ctx:
            persist = _ctx.enter_context(tc.tile_pool(name="persist", bufs=1))
            wpool2 = _ctx.enter_context(tc.tile_pool(name="wpool2", bufs=2))
            wpool1 = _ctx.enter_context(tc.tile_pool(name="wpool1", bufs=2))
            actp = _ctx.enter_context(tc.tile_pool(name="actp", bufs=1))
            xlnp = _ctx.enter_context(tc.tile_pool(name="xlnp", bufs=3))
            bigp = _ctx.enter_context(tc.tile_pool(name="bigp", bufs=3))
            expp = _ctx.enter_context(tc.tile_pool(name="expp", bufs=3))
            smallp = _ctx.enter_context(tc.tile_pool(name="smallp", bufs=3))
            tmpp = _ctx.enter_context(tc.tile_pool(name="tmpp", bufs=3))
            lgp = _ctx.enter_context(tc.tile_pool(name="lgp", bufs=4))
            whp = _ctx.enter_context(tc.tile_pool(name="whp", bufs=16))
            # PSUM: 'pair' 2x[128,2,TB] (4 banks) + 'mm' 2x (2) + 'op' 2x (2)
            ps_pair = _ctx.enter_context(tc.tile_pool(name="ps_pair", bufs=2, space="PSUM"))
            ps_mm = _ctx.enter_context(tc.tile_pool(name="ps_mm", bufs=2, space="PSUM"))
            ps_op = _ctx.enter_context(tc.tile_pool(name="ps_op", bufs=2, space="PSUM"))

            # single act-table preload: every later activation finds its
            # function already resident, so the compile pass inserts no
            # further loads
            nc.scalar.add_instruction(
                mybir.InstLoadActFuncSet(
                    name=nc.get_next_instruction_name(),
                    ins=[],
                    outs=[],
                    act_func_set_id=_act_table_id(nc.m.arch),
                )
            )

            # ---- persistent tiles ----
            xT = [persist.tile([128, t], BF16, tag=f"xT{e}", name=f"xT{e}") for e in range(ET)]
            for e in range(ET):
                nc.sync.dma_start(out=xT[e], in_=x0[128 * e : 128 * (e + 1), :])
            mask2b = persist.tile([SC, 2, 2 * SC], BF16, tag="mask2b")
            nc.sync.dma_start(out=mask2b, in_=mask2b_p[:, :, :])
            mask28 = persist.tile([SC, 2, 2 * SC], FP8, tag="mask28")
            nc.sync.dma_start(out=mask28, in_=mask28_p[:, :, :])
            # v tiles: per chunk cols = [vA(32) | ones | vB(32) | ones]; the
            # ones column makes row 32 of the AV output the softmax row-sum.
            # vt8: all chunks (fp8, DoubleRow path for t-blocks >= 1);
            # vtb: chunks 0-3 (bf16 path for t-block 0).
            # fp8 v tiles are 48 cols wide (dual-fp8 ldweights requires a
            # multiple of 16): [v(32) | ones | zeros(15)]
            VW = 48
            vt8s, vtbs = [], []
            for par in range(2):
                vt8 = persist.tile([128, nsc, 2, VW], FP8, tag=f"vt8_{par}", name=f"vt8_{par}")
                vtb = persist.tile([128, 4, 2, HD + 1], BF16, tag=f"vtb_{par}", name=f"vtb_{par}")
                nc.vector.memset(vt8[:, :, :, :], 0.0)
                nc.vector.memset(vt8[:, :, :, HD : HD + 1], 1.0)
                nc.vector.memset(vtb[:, :, :, HD : HD + 1], 1.0)
                vt8s.append(vt8)
                vtbs.append(vtb)
            # eyeblk holds 1/E so the stats matmuls yield mean and mean-square
            eyeblk = persist.tile([128, nt, nt], BF16, tag="eyeblk")
            nc.sync.dma_start(out=eyeblk, in_=peye[:, :, :])
            # own heads' normalized attention out, pre-AllGather, [32, t] each
            oTp = [persist.tile([HD, t], BF16, tag=f"oTp{h}", name=f"oTp{h}") for h in range(HPC)]
            epst = persist.tile([128, 1], FP32, tag="epst")
            nc.vector.memset(epst, EPS)

            def ln_stats_tb(src, st, tb):
                """x and x^2 1/E-sums for t-block tb. Even t-blocks use PSUM
                rows {0, 32}, odd rows {64, 96} (all matmul-position-legal),
                so each t-block's rstd chain reads 32-aligned rows and its
                outputs live at partition 0 for partition_broadcast."""
                r = tb % 2
                tbl = slice(TB * tb, TB * (tb + 1))
                for e in range(ET):
                    sq = bigp.tile([128, TB], BF16, tag="sq", name="sq")
                    if e == 0:
                        nc.vector.tensor_tensor(
                            out=sq, in0=src[e][:, tbl], in1=src[e][:, tbl], op=ALU.mult
                        )
                    else:
                        nc.scalar.activation(out=sq, in_=src[e][:, tbl], func=AF.Square)
                    nc.tensor.matmul(
                        st[64 * r : 64 * r + 1, :], eyeblk[:, tb, tb : tb + 1],
                        src[e][:, tbl],
                        start=(e == 0), stop=(e == ET - 1),
                        tile_position=(0, 64 * r),
                    )
                    nc.tensor.matmul(
                        st[32 + 64 * r : 33 + 64 * r, :], eyeblk[:, tb, tb : tb + 1],
                        sq,
                        start=(e == 0), stop=(e == ET - 1),
                        tile_position=(0, 32 + 64 * r),
                    )

            def ln_chain_apply_tb(src, st, tb, out_tiles):
                """rstd chain + scale/shift broadcast + apply for one t-block."""
                r = tb % 2
                var1 = smallp.tile([1, TB], FP32, tag="var4", name="var1")
                # mean^2 on ACT (a TensorTensor may read only ONE input from
                # PSUM, and both would be the stats bank here)
                nc.scalar.activation(
                    out=var1, in_=st[64 * r : 64 * r + 1, :], func=AF.Square
                )
                nc.vector.tensor_tensor(
                    out=var1, in0=st[32 + 64 * r : 33 + 64 * r, :], in1=var1,
                    op=ALU.subtract,
                )
                nc.scalar.activation(out=var1, in_=var1, func=AF.Ln, bias=epst[0:1, :])
                smt = smallp.tile([1, 2, TB], BF16, tag="smt", name="smt")
                nc.scalar.activation(out=smt[:, 0, :], in_=var1, func=AF.Exp, scale=-0.5)
                nc.vector.scalar_tensor_tensor(
                    out=smt[:, 1, :], in0=st[64 * r : 64 * r + 1, :], scalar=-1.0,
                    in1=smt[:, 0, :], op0=ALU.mult, op1=ALU.mult,
                )
                bc = tmpp.tile([128, 2, TB], BF16, tag="bc", name="bc")
                nc.gpsimd.partition_broadcast(bc[:, :, :], smt[0:1, :, :])
                for e in range(ET):
                    eng = nc.vector
                    tmp = bigp.tile([128, TB], BF16, tag="lnt2", name="lntmp2")
                    eng.tensor_tensor(
                        out=tmp, in0=src[e][:, TB * tb : TB * (tb + 1)],
                        in1=bc[:, 0, :], op=ALU.mult,
                    )
                    eng.tensor_tensor(
                        out=out_tiles[e][:, TB * tb : TB * (tb + 1)],
                        in0=tmp, in1=bc[:, 1, :], op=ALU.add,
                    )

            def layernorm(src, out_tiles):
                """Standalone per-half LN (used for layer 0's LN1)."""
                for half in range(2):
                    st = ps_mm.tile([98, TB], FP32, tag="mm", name="st")
                    for tb in (2 * half, 2 * half + 1):
                        ln_stats_tb(src, st, tb)
                        ln_chain_apply_tb(src, st, tb, out_tiles)

            # ---------- per-piece emitters ----------
            W = {}  # layer -> weight tiles

            def prefetch_weights(l):
                wq_t = [wpool2.tile([128, 6 * HD], BF16, tag=f"wqkv{e}", name=f"wqkv{e}") for e in range(ET)]
                wo_t = [wpool2.tile([128, E], BF16, tag=f"wo{e}", name=f"wot{e}") for e in range(ET)]
                w1_t = [wpool1.tile([128, FF], BF16, tag=f"w1{e}", name=f"w1t{e}") for e in range(ET)]
                w2_t = wpool1.tile([128, UT, E], BF16, tag="w2")
                vec = wpool2.tile([128, 20], FP32, tag="vec")
                for e in range(ET):
                    nc.sync.dma_start(out=wq_t[e], in_=wqkv[l, :, e, :])
                    nc.sync.dma_start(out=wo_t[e], in_=wo_p[l, :, e, :])
                    nc.sync.dma_start(out=w1_t[e], in_=w1_p[l, :, e, :])
                nc.sync.dma_start(out=w2_t, in_=w2_p[l, :, :, :])
                nc.sync.dma_start(out=vec, in_=vecs[l, :, :])
                W[l] = dict(wq=wq_t, wo=wo_t, w1=w1_t, w2=w2_t, vec=vec)

            def qkv_attn_tb(l, tb, xln, qT, kT, vt8, vtb):
                wq_t, vec = W[l]["wq"], W[l]["vec"]
                tsl = slice(TB * tb, TB * (tb + 1))
                # merged q|k projection: one 128-wide output per e
                qk = ps_mm.tile([128, TB], FP32, tag="mm", name="qk")
                for e in range(ET):
                    nc.tensor.matmul(
                        qk[:, :], wq_t[e][:, 0 : 4 * HD], xln[e][:, tsl],
                        start=(e == 0), stop=(e == ET - 1),
                    )
                # +cq/+ck: the ln1_b contributions, folded host-side
                nc.vector.tensor_scalar(
                    qT[:, tsl], qk[0 : 2 * HD, :], vec[0 : 2 * HD, 0:1], None, ALU.add
                )
                nc.scalar.activation(
                    out=kT[:, tsl], in_=qk[2 * HD : 4 * HD, :], func=AF.Identity,
                    bias=vec[2 * HD : 4 * HD, 0:1],
                )
                for i in range(4 * tb, 4 * tb + 4):
                    # vp lives in the 'mm' tag so op_ps (long-lived per
                    # t-block) only rotates against itself
                    vp = ps_mm.tile([128, 2, HD], FP32, tag="mm", name="vp")
                    for e in range(ET):
                        nc.tensor.matmul(
                            vp[:, :, :],
                            xln[e][:, SC * i : SC * (i + 1)],
                            wq_t[e][:, 4 * HD : 6 * HD],
                            start=(e == 0), stop=(e == ET - 1),
                        )
                    nc.vector.tensor_copy(out=vt8[:, i, :, 0:HD], in_=vp[:, :, :])
                    if i < 4:
                        nc.scalar.copy(out=vtb[:, i, :, 0:HD], in_=vp[:, :, :])

                # ---- attention for this t-block ----
                fp8blk = tb >= 1
                npair = 2 * tb + 2
                # one PSUM bank per head: dual-fp8 matmul output must start
                # at partition 0
                op_h = [ps_op.tile([128, TB], FP32, tag="op", name=f"op_h{h}")
                        for h in range(HPC)]

                def emit_av(j, h, exh, dp):
                    if fp8blk:
                        nc.tensor.matmul(
                            op_h[h][0:VW, dp:TB],
                            vt8[:, 2 * j : 2 * j + 2, h, :],
                            exh[:, :, dp:TB],
                            start=(j == 0), stop=(j == npair - 1),
                            perf_mode=DR,
                            tile_position=(0, 0),
                        )
                    else:
                        for jj in range(2):
                            nc.tensor.matmul(
                                op_h[h][0 : HD + 1, dp:TB],
                                vtb[:, 2 * j + jj, h, :],
                                exh[:, jj, dp:TB],
                                start=(j == 0 and jj == 0),
                                stop=(j == npair - 1 and jj == 1),
                                tile_position=(0, 0),
                            )

                exdt = FP8 if fp8blk else BF16
                m2 = mask28 if fp8blk else mask2b
                pend = []  # FIFO; AV trails exp by one pair
                for j in range(npair):
                    dp = max(0, SC * 2 * j - TB * tb)
                    psl = slice(dp, TB)
                    tgl = slice(TB * tb + dp, TB * (tb + 1))
                    for h in range(HPC):
                        rsl = slice(32 * h, 32 * (h + 1))
                        at_ps = ps_pair.tile([128, 2, TB], FP32, tag="pair", name="at_ps")
                        exh = expp.tile(
                            [128, 2, TB], exdt,
                            tag=f"exp{h}{'8' if fp8blk else 'b'}", name="exh",
                        )
                        for jj in range(2):
                            i = 2 * j + jj
                            nc.tensor.matmul(
                                at_ps[:, jj, psl],
                                kT[rsl, SC * i : SC * (i + 1)],
                                qT[rsl, tgl],
                                start=True, stop=True,
                                tile_position=(32 * h, 0),
                            )
                        nc.scalar.activation(
                            out=exh[:, :, psl], in_=at_ps[:, :, psl],
                            func=AF.Exp, scale=float(E) ** -0.5,
                        )
                        if j >= 2 * tb:  # diagonal pair: mask both chunks
                            nc.vector.tensor_tensor(
                                out=exh[:, :, dp : dp + 2 * SC],
                                in0=exh[:, :, dp : dp + 2 * SC],
                                in1=m2[:, :, :], op=ALU.mult,
                            )
                        pend.append((j, h, exh, dp))
                    while len(pend) > HPC:
                        emit_av(*pend.pop(0))
                for ent in pend:
                    emit_av(*ent)
                # normalize each head by its row-sum (psum row 64h+32):
                # bf16 reciprocal -> partition_broadcast -> multiply
                for h in range(HPC):
                    rr = smallp.tile([1, TB], BF16, tag=f"rr{h}", name="rr")
                    with nc.allow_low_precision(reason="1/rowsum applied in bf16"):
                        nc.vector.reciprocal(
                            out=rr[0:1, :],
                            in_=op_h[h][HD : HD + 1, :],
                        )
                    rb = tmpp.tile([HD, TB], BF16, tag="rbc", name="rb")
                    nc.gpsimd.partition_broadcast(rb[:, :], rr[0:1, :])
                    nc.vector.tensor_tensor(
                        out=oTp[h][:, TB * tb : TB * (tb + 1)],
                        in0=op_h[h][0:HD, :],
                        in1=rb,
                        op=ALU.mult,
                    )
                    # stream this t-block's slice to the AllGather bounce
                    # buffer now, so the collective input is ready the
                    # moment the last block finishes
                    nc.sync.dma_start(
                        out=cc_in[l][HD * h : HD * (h + 1), TB * tb : TB * (tb + 1)],
                        in_=oTp[h][:, TB * tb : TB * (tb + 1)],
                    )

            def wo_ln2(l, oT, xln2):
                """wo projection + residual, interleaved with the LN2 stats
                so the rstd chains hide under the following t-blocks' PE."""
                wo_t, vec = W[l]["wo"], W[l]["vec"]
                for half in range(2):
                    st = ps_mm.tile([98, TB], FP32, tag="mm", name="st2")
                    for tb in (2 * half, 2 * half + 1):
                        tsl = slice(TB * tb, TB * (tb + 1))
                        for eo in range(ET):
                            wp = ps_op.tile([128, TB], FP32, tag="op", name="wp")
                            for e in range(ET):
                                nc.tensor.matmul(
                                    wp[:, :],
                                    wo_t[e][:, 128 * eo : 128 * (eo + 1)],
                                    oT[e][:, tsl],
                                    start=(e == 0), stop=(e == ET - 1),
                                )
                            nc.vector.scalar_tensor_tensor(
                                out=xT[eo][:, tsl], in0=wp[:, :],
                                scalar=vec[:, 8 + eo : 9 + eo], in1=xT[eo][:, tsl],
                                op0=ALU.add, op1=ALU.add,
                            )
                        ln_stats_tb(xT, st, tb)
                        ln_chain_apply_tb(xT, st, tb, xln2)

            def ffn_lnnext(l, xln2, xnext):
                """FFN + residual, interleaved with the next LN's stats; the
                chains run while the other half's FFN matmuls execute."""
                w1_t, w2_t, vec = W[l]["w1"], W[l]["w2"], W[l]["vec"]
                for half in range(2):
                    st = ps_mm.tile([98, TB], FP32, tag="mm", name="st1")
                    for tb in (2 * half, 2 * half + 1):
                        tsl = slice(TB * tb, TB * (tb + 1))
                        ru_halves = []
                        for rh in range(2):
                            ru = bigp.tile([128, UT // 2, TB], BF16, tag="big", name="ru")
                            for uu in range(UT // 2):
                                ut = rh * (UT // 2) + uu
                                up = ps_op.tile([128, TB], FP32, tag="op", name="up")
                                for e in range(ET):
                                    nc.tensor.matmul(
                                        up[:, :],
                                        w1_t[e][:, 128 * ut : 128 * (ut + 1)],
                                        xln2[e][:, tsl],
                                        start=(e == 0), stop=(e == ET - 1),
                                    )
                                # relu+bias: split across ACT and Pool
                                if ut % 2 == 0:
                                    nc.scalar.activation(
                                        out=ru[:, uu, :], in_=up[:, :], func=AF.Relu,
                                        bias=vec[:, 10 + ut : 11 + ut],
                                    )
                                else:
                                    nc.vector.tensor_scalar(
                                        ru[:, uu, :], up[:, :],
                                        vec[:, 10 + ut : 11 + ut], 0.0,
                                        ALU.add, ALU.max,
                                    )
                            ru_halves.append(ru)
                        wp2_pair = ps_pair.tile([128, 2, TB], FP32, tag="pair", name="wp2")
                        for eo in range(ET):
                            wp2 = wp2_pair[:, eo % 2, :]
                            for ut in range(UT):
                                nc.tensor.matmul(
                                    wp2,
                                    w2_t[:, ut, 128 * eo : 128 * (eo + 1)],
                                    ru_halves[ut // (UT // 2)][:, ut % (UT // 2), :],
                                    start=(ut == 0), stop=(ut == UT - 1),
                                )
                            nc.vector.scalar_tensor_tensor(
                                out=xT[eo][:, tsl], in0=wp2,
                                scalar=vec[:, 18 + eo : 19 + eo], in1=xT[eo][:, tsl],
                                op0=ALU.add, op1=ALU.add,
                            )
                        ln_stats_tb(xT, st, tb)
                        ln_chain_apply_tb(xT, st, tb, xnext)

            GB = min(8, ntc)  # token-chunks batched per logits DMA
            whs = []

            def prefetch_whead():
                for vb in range(nvb):
                    wh = whp.tile([128, ET, 512], BF16, tag="wh", name=f"wh{vb}")
                    nc.sync.dma_start(out=wh, in_=whead[:, :, 512 * vb : 512 * (vb + 1)])
                    whs.append(wh)

            def lm_group(g, xf):
                for vb in range(nvb):
                    wh = whs[vb]
                    lg = lgp.tile([128, GB, 512], FP16, tag="lg")
                    lpp = None
                    for k in range(GB):
                        tcn = GB * g + k
                        # rotate lp across all 8 PSUM banks: mm, op, and the
                        # two halves of an (idle) attention pair tile
                        sel = k % 4
                        if sel == 0:
                            lp = ps_mm.tile([128, 512], FP32, tag="mm", name="lp")
                        elif sel == 1:
                            lp = ps_op.tile([128, 512], FP32, tag="op", name="lp")
                        else:
                            if sel == 2:
                                lpp = ps_pair.tile([128, 2, TB], FP32, tag="pair", name="lpp")
                            lp = lpp[:, sel - 2, :]
                        for e in range(ET):
                            nc.tensor.matmul(
                                lp,
                                xf[e][:, 128 * tcn : 128 * (tcn + 1)],
                                wh[:, e, :],
                                start=(e == 0), stop=(e == ET - 1),
                            )
                        if (vb + tcn) % 2 == 0:
                            nc.vector.tensor_copy(out=lg[:, k, :], in_=lp)
                        else:
                            nc.scalar.copy(out=lg[:, k, :], in_=lp)
                    nc.sync.dma_start(
                        out=logits[:, GB * g : GB * (g + 1), 512 * vb : 512 * (vb + 1)],
                        in_=lg,
                    )

            # ================= layers (software-pipelined) =================
            prefetch_weights(0)
            xln_cur = [xlnp.tile([128, t], BF16, tag=f"xln{e}", name=f"xln{e}") for e in range(ET)]
            layernorm(xT, out_tiles=xln_cur)
            for l in range(layers):
                if l + 1 < layers:
                    prefetch_weights(l + 1)
                else:
                    # lm_head weights stream in during the last layer so the
                    # first logits matmuls never wait on DMA
                    prefetch_whead()
                qT = actp.tile([2 * HD, t], BF16, tag="qT")
                kT = actp.tile([2 * HD, t], BF16, tag="kT")
                vt8, vtb = vt8s[l % 2], vtbs[l % 2]
                for tb in range(nt):
                    qkv_attn_tb(l, tb, xln_cur, qT, kT, vt8, vtb)

                # ---- AllGather heads across the 4-core batch group ----
                oT = [actp.tile([128, t], BF16, tag=tg, name=f"oT_{tg}") for tg in ("qT", "kT")]
                if use_collective:
                    nc.gpsimd.collective_compute(
                        "AllGather", ALU.bypass,
                        replica_groups=groups,
                        ins=[cc_in[l][:, :]], outs=[cc_out[l][:, :]],
                    )
                # per-t-block loads so wo(tb0) unblocks before the full
                # gather output lands (no-collective build: same DMAs,
                # timing-only)
                for e in range(ET):
                    for tb in range(nt):
                        nc.sync.dma_start(
                            out=oT[e][:, TB * tb : TB * (tb + 1)],
                            in_=cc_out[l][128 * e : 128 * (e + 1), TB * tb : TB * (tb + 1)],
                        )

                xln2 = [xlnp.tile([128, t], BF16, tag=f"xln{e}", name=f"xln{e}") for e in range(ET)]
                wo_ln2(l, oT, xln2)
                xnext = [xlnp.tile([128, t], BF16, tag=f"xln{e}", name=f"xln{e}") for e in range(ET)]
                ffn_lnnext(l, xln2, xnext)
                xln_cur = xnext

            # ================= lm_head =================
            lm_group(0, xln_cur)
            lm_group(1, xln_cur)
    nc.compile()
    return nc


# ---------------- host-side prep / unshard ----------------

def prep_core_inputs(c, X, tok_emb, pos_emb, wq, wk, wv, wo, bo, w1, b1, w2, b2,
                     ln1_g, ln1_b, ln2_g, ln2_b, lnf_g, lnf_b, w_head, b_head,
                     t=T, layers=L, vsp=VSP):
    b = c // GROUP
    j = c % GROUP
    heads = [HPC * j + k for k in range(HPC)]

    f32 = np.float32
    Xb = np.asarray(X[b]).astype(np.int64)
    x0 = (np.asarray(tok_emb)[Xb] + np.asarray(pos_emb)[:t]).astype(f32).T  # [E, t]

    wq = np.asarray(wq); wk = np.asarray(wk); wv = np.asarray(wv)
    wqkv_h = np.empty((layers, 128, ET, 6 * HD), f32)
    wo_h = np.empty((layers, 128, ET, E), f32)
    w1_h = np.empty((layers, 128, ET, FF), f32)
    w2_h = np.empty((layers, 128, UT, E), f32)
    vecs_h = np.zeros((layers, 128, 20), f32)
    for l in range(layers):
        # fold LN gains into the consuming weights and LN biases into
        # per-output-constant corrections (exact for affine LN):
        #   xln_true = xln_raw * g + b  =>  W^T xln_true = (gW)^T xln_raw + W^T b
        g1 = np.asarray(ln1_g[l]).astype(f32)[:, None]
        b1n = np.asarray(ln1_b[l]).astype(f32)
        g2 = np.asarray(ln2_g[l]).astype(f32)[:, None]
        b2n = np.asarray(ln2_b[l]).astype(f32)
        qc = np.concatenate([wq[l, h] for h in heads], axis=1)  # [E, 64]
        kc = np.concatenate([wk[l, h] for h in heads], axis=1)
        vc = np.concatenate([wv[l, h] for h in heads], axis=1)
        cq = qc.T @ b1n  # [64] q bias from ln1_b
        ck = kc.T @ b1n
        # v bias from ln1_b for ALL heads, folded through wo into bo
        cv_full = np.concatenate([wv[l, h].T @ b1n for h in range(H)])  # [E]
        bo_eff = np.asarray(bo[l]).astype(f32) + np.asarray(wo[l]).T @ cv_full
        b1_eff = np.asarray(b1[l]).astype(f32) + np.asarray(w1[l]).T @ b2n
        qkv = np.concatenate([qc, kc, vc], axis=1) * g1  # [E, 192]
        wqkv_h[l] = qkv.reshape(ET, 128, 6 * HD).transpose(1, 0, 2)
        wo_h[l] = np.asarray(wo[l]).reshape(ET, 128, E).transpose(1, 0, 2)
        w1_h[l] = (np.asarray(w1[l]) * g2).reshape(ET, 128, FF).transpose(1, 0, 2)
        w2_h[l] = np.asarray(w2[l]).reshape(UT, 128, E).transpose(1, 0, 2)
        vecs_h[l, 0 : 2 * HD, 0] = cq
        vecs_h[l, 2 * HD : 4 * HD, 0] = ck
        vecs_h[l, :, 8:10] = bo_eff.reshape(2, 128).T
        vecs_h[l, :, 10:18] = b1_eff.reshape(8, 128).T
        vecs_h[l, :, 18:20] = np.asarray(b2[l]).astype(f32).reshape(2, 128).T

    w_head = np.asarray(w_head) * np.asarray(lnf_g).astype(f32)[:, None]
    vs = w_head.shape[1] // GROUP
    wh = np.zeros((E, vsp), f32)
    wh[:, :vs] = w_head[:, vs * j : vs * (j + 1)]
    whead_h = np.ascontiguousarray(wh.reshape(ET, 128, vsp).transpose(1, 0, 2))

    # diag-pair mask [s, 2, 2*SC]: chunk-lo gets [tri | ones], chunk-hi
    # (one chunk later) gets [zeros | tri]
    sp = np.arange(SC)[:, None]
    cp = np.arange(2 * SC)[None, :]
    mask2 = np.zeros((SC, 2, 2 * SC), f32)
    mask2[:, 0, :] = (sp <= cp).astype(f32)
    mask2[:, 1, :] = (sp <= cp - SC).astype(f32)

    nt = t // TB
    peye_h = np.zeros((128, nt, nt), f32)
    for tb in range(nt):
        peye_h[:, tb, tb] = 1.0 / E

    bf = NP_BF16
    return {
        "x0": np.ascontiguousarray(x0).astype(bf),
        "wqkv": np.ascontiguousarray(wqkv_h).astype(bf),
        "wo": np.ascontiguousarray(wo_h).astype(bf),
        "w1": np.ascontiguousarray(w1_h).astype(bf),
        "w2": np.ascontiguousarray(w2_h).astype(bf),
        "vecs": np.ascontiguousarray(vecs_h),
        "whead": whead_h.astype(bf),
        "mask2b": mask2.astype(bf),
        "mask28": mask2.astype(NP_FP8),
        "peye": peye_h.astype(bf),
    }


_NC_CACHE = {}


def _get_nc():
    if "nc" not in _NC_CACHE:
        _NC_CACHE["nc"] = build_nc()
    return _NC_CACHE["nc"]


def kernel(**inputs):
    nc = _get_nc()
    in_maps = [prep_core_inputs(c, **inputs) for c in range(NCORES)]
    res = run_bass_kernel_spmd(nc, in_maps, list(range(NCORES)))
    out = np.empty((B, T, V), np.float32)
    for c in range(NCORES):
        b, j = c // GROUP, c % GROUP
        lg = res.results[c]["logits"]  # [128, T//128, VSP]
        lg = lg.transpose(1, 0, 2).reshape(T, VSP)
        out[b, :, VS * j : VS * (j + 1)] = lg[:, :VS].astype(np.float32)
    # b_head plus the final-LN bias folded through w_head (host-side)
    bh_eff = np.asarray(inputs["b_head"]).astype(np.float32) + (
        np.asarray(inputs["w_head"]).astype(np.float32).T
        @ np.asarray(inputs["lnf_b"]).astype(np.float32)
    )
    if np.any(bh_eff):
        out += bh_eff[None, None, :]
    return out
